# revision 7
# baseline (speedup 1.0000x reference)
"""nn_AttentionOut_63711544869147 — causal multi-head attention + output projection,
distributed over 8 Trainium2 NeuronCores.

Module: out = softmax(causal(Q K^T / sqrt(d))) V @ W_O + b_O, returned with the
(unchanged) residual: reference returns the tuple (residual, out).

Sharding (8 cores = 2 batches x 4 head-groups of 4 heads, SPMD single program):
  each core computes full causal attention for its batch over its 4 heads and
  a partial projection  sum_{h in group} z_h @ W_O[h]  ->  pout [2048, 1024].
  The host sums the 4 head-group partials per batch (the "all-reduce" of the
  row-sharded W_O product), adds b_O, and passes the residual through.

Device dataflow per (head, 512-wide q strip), exact causal tiling:
  scores_T[kv,q] = K_h^T_tile.T @ Q_h^T_strip        (PE, bf16, N=512 cols out)
  expP = exp(scores_T * 1/8)                          (ACT; 1/8 folds 1/sqrt(64);
                                                       kv-tile PAIRS share one
                                                       exp instr; the 4 diagonal
                                                       tiles pack their partial
                                                       ranges into 2 exp instrs)
  causal masks on the leading 128-col blocks          (GPSIMD tensor_mul — keeps
                                                       DVE free; all-SBUF bf16)
  z_ext[128,q] += V_ext_tile.T @ expP                 (PE accum; V_ext columns =
                                                       [V|1] even heads, [1|V]
                                                       odd heads, so z and the
                                                       softmax denominator land
                                                       on opposite partition
                                                       halves — engines are
                                                       partition-locked)
  zn = z * (1/denom)   (DVE copy+recip on the denom half, GPSIMD
                        partition_broadcast shifts 1/denom onto z's half,
                        DVE mul; no DRAM round-trip)
  pout strip = zn_T @ W_O_group                       (PE, 2x128-deep accum)

PE emission is software-pipelined: each PV pair is deferred until after the
NEXT pair's QK+exp is emitted (one-pair lookahead keeps the in-order PE queue
from stalling on ACT), and the previous strip's projection matmuls are
interleaved at head boundaries as PE filler during ACT-bound stretches. This
keeps the PE p-state ramp warm (2.4 GHz needs >3us continuous busy).
"""

import numpy as np

import concourse.bass as bass
import concourse.bacc as bacc
import concourse.tile as tile
from concourse import mybir
from concourse.bass_utils import run_bass_kernel_spmd

F32 = mybir.dt.float32
BF16 = mybir.dt.bfloat16

N_CORES = 8
N_HEADS = 16
H = 4          # heads per core
S = 2048
D = 64
P = 128
D_MODEL = 1024
NSTRIP = 4     # q strips of 512
QW = 512       # strip width

_PROGRAM = None
LAST_RESULTS = None


def build_program():
    MMDT = BF16
    nc = bacc.Bacc(target_bir_lowering=False)

    qT = nc.dram_tensor("qT", [H, D, S], MMDT, kind="ExternalInput")
    kT = nc.dram_tensor("kT", [H, D, S], MMDT, kind="ExternalInput")
    # v prearranged by host: [head, partition(kv%128), kvtile(16), d(64)]
    v = nc.dram_tensor("v", [H, P, 16 * D], MMDT, kind="ExternalInput")
    wo = nc.dram_tensor("wo", [2 * P, D_MODEL], MMDT, kind="ExternalInput")
    tri = nc.dram_tensor("tri", [P, P], MMDT, kind="ExternalInput")
    pout = nc.dram_tensor("pout", [S, D_MODEL], MMDT, kind="ExternalOutput")

    with tile.TileContext(nc) as tc:
        with (
            tc.tile_pool(name="persist", bufs=1) as persist,
            tc.tile_pool(name="expp", bufs=4) as expp,
            tc.tile_pool(name="nrm", bufs=2) as nrm,
            tc.tile_pool(name="outp", bufs=4) as outp,
            tc.tile_pool(name="znp", bufs=2) as znp,
            tc.tile_pool(name="scps", bufs=2, space="PSUM") as scps,
            tc.tile_pool(name="zps", bufs=2, space="PSUM") as zps,
            tc.tile_pool(name="wops", bufs=2, space="PSUM") as wops,
        ):
            # ---- persistent tiles (declared up front, loaded in need order) ----
            # kT/qT split so the first matmul only waits for its own slice.
            kT_sb = [[persist.tile([P, S // 2], MMDT, tag=f"kT{j}{half}",
                                   name=f"kT{j}{half}")
                      for half in range(2)] for j in range(2)]
            qT_sb = [[persist.tile([P, QW], MMDT, tag=f"qT{j}{s}",
                                   name=f"qT{j}{s}")
                      for s in range(NSTRIP)] for j in range(2)]
            # vext: per kv tile t the 128-wide block is [v_t | ones] for even
            # heads, [ones | v_t] for odd heads -> z lands on partitions 0:64
            # (even) / 64:128 (odd), softmax denominator on the other half.
            # (interleaved per tile: matmul weights AP needs ONE free dim)
            vext_sb = [persist.tile([P, 16, P], MMDT, tag=f"vext{h}",
                                    name=f"vext{h}") for h in range(H)]
            wo_sb = [persist.tile([P, D_MODEL], MMDT, tag=f"wo{j}",
                                  name=f"wo{j}") for j in range(2)]
            tri_sb = persist.tile([P, P], MMDT, tag="tri", name="tri_sb")

            # need-ordered loads (the cost model serializes all DMA transfers
            # on one resource, so emission order is load order)
            nc.sync.dma_start(kT_sb[0][0][:], kT[0:2, :, 0 : S // 2].rearrange("h d s -> (h d) s"))
            nc.sync.dma_start(qT_sb[0][0][:], qT[0:2, :, 0:QW].rearrange("h d s -> (h d) s"))
            nc.sync.dma_start(tri_sb[:], tri[:])
            for h in (0, 1):
                vlo = 0 if h % 2 == 0 else D
                nc.gpsimd.memset(vext_sb[h][:, :, D - vlo : P - vlo], 1.0)
                nc.sync.dma_start(vext_sb[h][:, :, vlo : vlo + D], v[h])
            nc.sync.dma_start(qT_sb[0][1][:], qT[0:2, :, QW : 2 * QW].rearrange("h d s -> (h d) s"))
            nc.sync.dma_start(kT_sb[1][0][:], kT[2:4, :, 0 : S // 2].rearrange("h d s -> (h d) s"))
            for h in (2, 3):
                vlo = 0 if h % 2 == 0 else D
                nc.gpsimd.memset(vext_sb[h][:, :, D - vlo : P - vlo], 1.0)
                nc.sync.dma_start(vext_sb[h][:, :, vlo : vlo + D], v[h])
            nc.sync.dma_start(qT_sb[1][0][:], qT[2:4, :, 0:QW].rearrange("h d s -> (h d) s"))
            nc.sync.dma_start(qT_sb[1][1][:], qT[2:4, :, QW : 2 * QW].rearrange("h d s -> (h d) s"))
            nc.sync.dma_start(kT_sb[0][1][:], kT[0:2, :, S // 2 : S].rearrange("h d s -> (h d) s"))
            nc.sync.dma_start(kT_sb[1][1][:], kT[2:4, :, S // 2 : S].rearrange("h d s -> (h d) s"))
            for j in range(2):
                nc.sync.dma_start(wo_sb[j][:], wo[P * j : P * (j + 1), :])
            for s in (2, 3):
                for j in range(2):
                    nc.sync.dma_start(qT_sb[j][s][:], qT[2 * j : 2 * j + 2, :, s * QW : (s + 1) * QW].rearrange("h d s -> (h d) s"))

            def kslice(j, off, t):
                """kT lhsT slice for head row range [off:off+64), kv tile t."""
                half, col = t // 8, (t % 8) * P
                return kT_sb[j][half][off : off + D, col : col + P]

            def vslice(h, t):
                """vext lhsT [128, 128] free = [v_t|ones] / [ones|v_t]."""
                return vext_sb[h][:, t, :]

            zn_sb = {}     # (strip, j) -> zn tile
            ot_sb = {}

            def emit_proj_units(sp, units):
                """Projection for strip sp, q-blocks/mt pairs in `units`."""
                for qb, mt in units:
                    ops = wops.tile([P, 512], F32, tag="wo_ps", name="wo_ps")
                    for j2 in range(2):
                        nc.tensor.matmul(
                            ops[:],
                            zn_sb[(sp, j2)][:, qb * P : (qb + 1) * P],
                            wo_sb[j2][:, mt * 512 : (mt + 1) * 512],
                            start=(j2 == 0),
                            stop=(j2 == 1),
                        )
                    if mt == 0:
                        ot_sb[(sp, qb)] = outp.tile([P, D_MODEL], MMDT, tag="ot", name="ot")
                    nc.vector.tensor_copy(ot_sb[(sp, qb)][:, mt * 512 : (mt + 1) * 512], ops[:])
                    if mt == 1:
                        nc.sync.dma_start(
                            pout[(4 * sp + qb) * P : (4 * sp + qb + 1) * P, :],
                            ot_sb[(sp, qb)][:],
                        )

            # ---- main loops ----
            for s in range(NSTRIP):
                q0 = s * QW
                for j in range(2):
                    zn_sb[(s, j)] = znp.tile([P, QW], MMDT, tag=f"zn{j}", name=f"zn{j}")
                deferred_pv = None  # one-pair PE lookahead

                for h in range(H):
                    j, off = h // 2, (h % 2) * D
                    odd = h % 2
                    z_ps = zps.tile([P, QW], F32, tag="z", name="z_ps")
                    qs = qT_sb[j][s]

                    # pair list: 2s full pairs + diag A (d0,d1) + diag B (d2,d3)
                    # each entry emits its QK+exp(+mask) now; PV is deferred.
                    pairs = [("full", 2 * p) for p in range(2 * s)]
                    pairs.append(("dA", 4 * s))
                    pairs.append(("dB", 4 * s + 2))

                    for kind, t0 in pairs:
                        sc = scps.tile([P, 2, QW], F32, tag="sc", name="sc")
                        ex = expp.tile([P, 2, QW], MMDT, tag="ex", name="ex")
                        scf = sc[:].rearrange("p a b -> p (a b)")
                        exf = ex[:].rearrange("p a b -> p (a b)")
                        if kind == "full":
                            for o in (0, 1):
                                nc.tensor.matmul(
                                    sc[:, o, :], kslice(j, off, t0 + o),
                                    qs[off : off + D, :], start=True, stop=True,
                                )
                            nc.scalar.activation(
                                exf[:, 0:1024], scf[:, 0:1024],
                                mybir.ActivationFunctionType.Exp, scale=0.125,
                            )
                            pv = [(t0, exf[:, 0:512], 0), (t0 + 1, exf[:, 512:1024], 0)]
                        elif kind == "dA":
                            # d0: q cols [0:512); d1: q cols [128:512) packed at 512
                            nc.tensor.matmul(
                                scf[:, 0:512], kslice(j, off, t0),
                                qs[off : off + D, :], start=True, stop=True,
                            )
                            nc.tensor.matmul(
                                scf[:, 512:896], kslice(j, off, t0 + 1),
                                qs[off : off + D, P:QW], start=True, stop=True,
                            )
                            nc.scalar.activation(
                                exf[:, 0:896], scf[:, 0:896],
                                mybir.ActivationFunctionType.Exp, scale=0.125,
                            )
                            nc.gpsimd.tensor_mul(exf[:, 0:P], exf[:, 0:P], tri_sb[:])
                            nc.gpsimd.tensor_mul(exf[:, 512 : 512 + P], exf[:, 512 : 512 + P], tri_sb[:])
                            pv = [(t0, exf[:, 0:512], 0), (t0 + 1, exf[:, 512:896], P)]
                        else:  # dB
                            # d2: q cols [256:512); d3: q cols [384:512)
                            nc.tensor.matmul(
                                scf[:, 0:256], kslice(j, off, t0),
                                qs[off : off + D, 2 * P : QW], start=True, stop=True,
                            )
                            nc.tensor.matmul(
                                scf[:, 256:384], kslice(j, off, t0 + 1),
                                qs[off : off + D, 3 * P : QW], start=True, stop=True,
                            )
                            nc.scalar.activation(
                                exf[:, 0:384], scf[:, 0:384],
                                mybir.ActivationFunctionType.Exp, scale=0.125,
                            )
                            nc.gpsimd.tensor_mul(exf[:, 0:P], exf[:, 0:P], tri_sb[:])
                            nc.gpsimd.tensor_mul(exf[:, 256:384], exf[:, 256:384], tri_sb[:])
                            pv = [(t0, exf[:, 0:256], 2 * P), (t0 + 1, exf[:, 256:384], 3 * P)]

                        if deferred_pv is not None:
                            for (tt, rhs, qoff), zt, first, last in deferred_pv:
                                nc.tensor.matmul(
                                    zt[1][:, qoff:QW] if qoff else zt[1][:],
                                    vslice(zt[0], tt), rhs,
                                    start=first, stop=last,
                                )
                        deferred_pv = [
                            (pv[0], (h, z_ps), t0 == 0, False),
                            (pv[1], (h, z_ps), False, kind == "dB"),
                        ]

                    # flush this head's last PV before the normalize chain
                    # that reads z_ps.
                    for (tt, rhs, qoff), zt, first, last in deferred_pv:
                        nc.tensor.matmul(
                            zt[1][:, qoff:QW] if qoff else zt[1][:],
                            vslice(zt[0], tt), rhs,
                            start=first, stop=last,
                        )
                    deferred_pv = None

                    # normalize: z/denom. even head: z rows 0:64, denom rows
                    # 64:128; odd head: swapped (vext column order picks it).
                    zlo, zhi = (0, D) if not odd else (D, P)
                    dlo = D if not odd else 0
                    den = nrm.tile([P, QW], F32, tag="den", name="den")
                    rb = nrm.tile([P, QW], F32, tag="rb", name="rb")
                    # copy one denominator row to SBUF (recip needs SBUF in)
                    nc.vector.tensor_copy(den[dlo : dlo + 1, :], z_ps[dlo : dlo + 1, :])
                    nc.vector.reciprocal_approx_fast(rb[dlo : dlo + 1, :], den[dlo : dlo + 1, :])
                    # shift+broadcast 1/denom onto z's partition half (GPSIMD
                    # daisy chain — the only engine that can cross partitions)
                    nc.gpsimd.partition_broadcast(rb[zlo:zhi, :], rb[dlo : dlo + 1, :], channels=D)
                    nc.vector.tensor_mul(
                        zn_sb[(s, j)][off : off + D, :], z_ps[zlo:zhi, :], rb[zlo:zhi, :]
                    )

                    # PE filler at head boundaries: previous strip's projection
                    if s > 0:
                        if h == 1:
                            emit_proj_units(s - 1, [(0, 0), (0, 1), (1, 0), (1, 1)])
                        elif h == 2:
                            emit_proj_units(s - 1, [(2, 0), (2, 1), (3, 0), (3, 1)])

            # tail: last strip's projection
            emit_proj_units(NSTRIP - 1, [(qb, mt) for qb in range(4) for mt in range(2)])

    nc.finalize()
    return nc


def _get_program():
    global _PROGRAM
    if _PROGRAM is None:
        _PROGRAM = build_program()
    return _PROGRAM


def make_in_maps(q, k, v, W_O, n_cores=N_CORES):
    """Shard full inputs into per-core maps (core = batch*4 + head_group)."""
    import ml_dtypes
    mmdt = ml_dtypes.bfloat16
    q = np.ascontiguousarray(np.asarray(q, dtype=np.float32))
    k = np.ascontiguousarray(np.asarray(k, dtype=np.float32))
    v = np.ascontiguousarray(np.asarray(v, dtype=np.float32))
    W_O = np.ascontiguousarray(np.asarray(W_O, dtype=np.float32))
    B = q.shape[0]
    qT = np.ascontiguousarray(q.reshape(B, S, N_HEADS, D).transpose(0, 2, 3, 1))
    kT = np.ascontiguousarray(k.reshape(B, S, N_HEADS, D).transpose(0, 2, 3, 1))
    # v: [b, h, kvtile(16), p(128), d] -> [b, h, p, (t d)]
    vh = v.reshape(B, S, N_HEADS, D).transpose(0, 2, 1, 3)  # [b, h, S, d]
    vh = vh.reshape(B, N_HEADS, 16, P, D).transpose(0, 1, 3, 2, 4)  # [b,h,p,t,d]
    vh = np.ascontiguousarray(vh.reshape(B, N_HEADS, P, 16 * D))
    # mask[kv, q] = 1 iff kv <= q  (scores live transposed: partition=kv, free=q)
    tri = np.ascontiguousarray(np.triu(np.ones((P, P), dtype=np.float32)))
    in_maps = []
    for core in range(n_cores):
        b, g = core // 4, core % 4
        hs = slice(H * g, H * (g + 1))
        in_maps.append(
            {
                "qT": np.ascontiguousarray(qT[b, hs]).astype(mmdt),
                "kT": np.ascontiguousarray(kT[b, hs]).astype(mmdt),
                "v": np.ascontiguousarray(vh[b, hs]).astype(mmdt),
                "wo": np.ascontiguousarray(W_O[hs].reshape(2 * P, D_MODEL)).astype(mmdt),
                "tri": tri.astype(mmdt),
            }
        )
    return in_maps


def kernel(residual, q, k, v, W_O, b_O, _trace=False, _trace_kwargs=None):
    global LAST_RESULTS
    residual = np.asarray(residual, dtype=np.float32)
    B = residual.shape[0]
    in_maps = make_in_maps(q, k, v, W_O)
    nc = _get_program()
    res = run_bass_kernel_spmd(
        nc, in_maps, list(range(N_CORES)), trace=_trace, **(_trace_kwargs or {})
    )
    LAST_RESULTS = res
    out = np.zeros((B, S, D_MODEL), dtype=np.float64)
    for core in range(N_CORES):
        out[core // 4] += res.results[core]["pout"].astype(np.float64)
    out += np.asarray(b_O, dtype=np.float64)
    return (residual, out.astype(np.float32))


# revision 10
# speedup vs baseline: 2.1581x; 2.1581x over previous
"""nn_AttentionOut_63711544869147 — causal multi-head attention + output projection,
distributed over 8 Trainium2 NeuronCores.

Module: out = softmax(causal(Q K^T / sqrt(d))) V @ W_O + b_O, returned with the
(unchanged) residual: reference returns the tuple (residual, out).

Sharding (8 cores = 2 batches x 4 head-groups of 4 heads, SPMD single program):
  each core computes full causal attention for its batch over its 4 heads and
  a partial projection  sum_{h in group} z_h @ W_O[h]  ->  pout [2048, 1024].
  The host sums the 4 head-group partials per batch (the "all-reduce" of the
  row-sharded W_O product), adds b_O, and passes the residual through.

Device dataflow per (head, 512-wide q strip), exact causal tiling:
  scores_T[kv,q] = K_h^T_tile.T @ Q_h^T_strip      (PE bf16; kv-tile pairs into
                                                    a 2-bank PSUM tile)
  expP = exp(scores_T * 1/8)                        (ACT; scale folds 1/sqrt(64);
                                                    one exp per kv-tile pair;
                                                    diagonal tiles pack their
                                                    partial ranges into 2 tiles)
  triangular mask on diagonal blocks                (DVE mul by 0/1 matrix)
  z_ext[65,q] += V_ext_tile.T @ expP                (PE accum; V_ext = [V | 1]
                                                    so row 64 = softmax denom)
  zn = z[0:64] * (1/z[64]) (DVE copy + approx-recip + DRAM-bounce broadcast
                            + mul; DVE crossbar allows the partition-offset
                            write for odd heads)
  pout strip = zn_T @ W_O_group                     (PE, 2x128-deep contraction)

PE emission is software-pipelined: each PV pair is deferred until after the
NEXT pair's QK+exp is emitted (one-pair lookahead keeps the in-order PE queue
from stalling on ACT), and the previous strip's projection matmuls are
interleaved at head boundaries as PE filler during ACT-bound stretches. This
keeps the PE p-state ramp warm (2.4 GHz needs >3us continuous busy). Input
DMAs are split and emitted in need order (the cost model serializes all DMA
transfers on one resource).
"""

import numpy as np

import concourse.bass as bass
import concourse.bacc as bacc
import concourse.tile as tile
from concourse import mybir
from concourse.bass_utils import run_bass_kernel_spmd

F32 = mybir.dt.float32
BF16 = mybir.dt.bfloat16

N_CORES = 8
N_HEADS = 16
H = 4          # heads per core
S = 2048
D = 64
P = 128
D_MODEL = 1024
NSTRIP = 4     # q strips of 512
QW = 512       # strip width

# feature flags (conservative=False values use only baseline-proven constructs)
PACK_DIAG = True        # pack the 4 diagonal tiles into 2 sc tiles / 2 exps
USE_GPS_MASK = False    # tri masks on gpsimd vs DVE
USE_GPS_BCAST = False   # gpsimd partition_broadcast vs DRAM-bounce broadcast
USE_BF16_OUT = False    # pout in bf16 (halves output DMA) vs f32

_PROGRAM = None
_PROGRAM_KEY = None
LAST_RESULTS = None


def build_program():
    MMDT = BF16
    ODT = BF16 if USE_BF16_OUT else F32
    nc = bacc.Bacc(target_bir_lowering=False)

    qT = nc.dram_tensor("qT", [H, D, S], MMDT, kind="ExternalInput")
    kT = nc.dram_tensor("kT", [H, D, S], MMDT, kind="ExternalInput")
    # v prearranged by host: [head, partition(kv%128), kvtile(16), d(64)]
    v = nc.dram_tensor("v", [H, P, 16 * D], MMDT, kind="ExternalInput")
    wo = nc.dram_tensor("wo", [2 * P, D_MODEL], MMDT, kind="ExternalInput")
    tri = nc.dram_tensor("tri", [P, P], MMDT, kind="ExternalInput")
    pout = nc.dram_tensor("pout", [S, D_MODEL], ODT, kind="ExternalOutput")

    MASKENG = nc.gpsimd if USE_GPS_MASK else nc.vector

    with tile.TileContext(nc) as tc:
        with (
            tc.tile_pool(name="persist", bufs=1) as persist,
            tc.tile_pool(name="expp", bufs=4) as expp,
            tc.tile_pool(name="rcpp", bufs=2) as rcpp,
            tc.tile_pool(name="outp", bufs=4) as outp,
            tc.tile_pool(name="znp", bufs=2) as znp,
            tc.tile_pool(name="scps", bufs=2, space="PSUM") as scps,
            tc.tile_pool(name="zps", bufs=2, space="PSUM") as zps,
            tc.tile_pool(name="wops", bufs=2, space="PSUM") as wops,
            tc.tile_pool(name="dramp", bufs=2, space="DRAM") as dramp,
        ):
            # ---- persistent tiles; loads emitted in need order ----
            kT_sb = [[persist.tile([P, S // 2], MMDT, tag=f"kT{j}{half}",
                                   name=f"kT{j}{half}")
                      for half in range(2)] for j in range(2)]
            qT_sb = [[persist.tile([P, QW], MMDT, tag=f"qT{j}{s}",
                                   name=f"qT{j}{s}")
                      for s in range(NSTRIP)] for j in range(2)]
            # V_ext = [V | 1]: row 64 of the PV product is the softmax denom
            vext_sb = [persist.tile([P, 16, D + 1], MMDT, tag=f"vext{h}",
                                    name=f"vext{h}") for h in range(H)]
            wo_sb = [persist.tile([P, D_MODEL], MMDT, tag=f"wo{j}",
                                  name=f"wo{j}") for j in range(2)]
            tri_sb = persist.tile([P, P], MMDT, tag="tri", name="tri_sb")

            nc.sync.dma_start(kT_sb[0][0][:], kT[0:2, :, 0 : S // 2].rearrange("h d s -> (h d) s"))
            nc.sync.dma_start(qT_sb[0][0][:], qT[0:2, :, 0:QW].rearrange("h d s -> (h d) s"))
            nc.sync.dma_start(tri_sb[:], tri[:])
            for h in (0, 1):
                nc.vector.memset(vext_sb[h][:, :, D : D + 1], 1.0)
                nc.sync.dma_start(
                    vext_sb[h][:, :, 0:D],
                    v[h].rearrange("p (t d) -> p t d", d=D),
                )
            nc.sync.dma_start(qT_sb[0][1][:], qT[0:2, :, QW : 2 * QW].rearrange("h d s -> (h d) s"))
            nc.sync.dma_start(kT_sb[1][0][:], kT[2:4, :, 0 : S // 2].rearrange("h d s -> (h d) s"))
            for h in (2, 3):
                nc.vector.memset(vext_sb[h][:, :, D : D + 1], 1.0)
                nc.sync.dma_start(
                    vext_sb[h][:, :, 0:D],
                    v[h].rearrange("p (t d) -> p t d", d=D),
                )
            nc.sync.dma_start(qT_sb[1][0][:], qT[2:4, :, 0:QW].rearrange("h d s -> (h d) s"))
            nc.sync.dma_start(qT_sb[1][1][:], qT[2:4, :, QW : 2 * QW].rearrange("h d s -> (h d) s"))
            nc.sync.dma_start(kT_sb[0][1][:], kT[0:2, :, S // 2 : S].rearrange("h d s -> (h d) s"))
            nc.sync.dma_start(kT_sb[1][1][:], kT[2:4, :, S // 2 : S].rearrange("h d s -> (h d) s"))
            for j in range(2):
                nc.sync.dma_start(wo_sb[j][:], wo[P * j : P * (j + 1), :])
            for s in (2, 3):
                for j in range(2):
                    nc.sync.dma_start(qT_sb[j][s][:], qT[2 * j : 2 * j + 2, :, s * QW : (s + 1) * QW].rearrange("h d s -> (h d) s"))

            def kslice(j, off, t):
                half, col = t // 8, (t % 8) * P
                return kT_sb[j][half][off : off + D, col : col + P]

            zn_sb = {}     # (strip, j) -> zn tile
            ot_sb = {}

            def emit_proj_units(sp, units):
                """Projection for strip sp over (q-block, mt-half) units."""
                for qb, mt in units:
                    ops = wops.tile([P, 512], F32, tag="wo_ps", name="wo_ps")
                    for j2 in range(2):
                        nc.tensor.matmul(
                            ops[:],
                            zn_sb[(sp, j2)][:, qb * P : (qb + 1) * P],
                            wo_sb[j2][:, mt * 512 : (mt + 1) * 512],
                            start=(j2 == 0),
                            stop=(j2 == 1),
                        )
                    if mt == 0:
                        ot_sb[(sp, qb)] = outp.tile([P, D_MODEL], ODT, tag="ot", name="ot")
                    nc.vector.tensor_copy(ot_sb[(sp, qb)][:, mt * 512 : (mt + 1) * 512], ops[:])
                    if mt == 1:
                        nc.sync.dma_start(
                            pout[(4 * sp + qb) * P : (4 * sp + qb + 1) * P, :],
                            ot_sb[(sp, qb)][:],
                        )

            # ---- main loops ----
            for s in range(NSTRIP):
                for j in range(2):
                    zn_sb[(s, j)] = znp.tile([P, QW], MMDT, tag=f"zn{j}", name=f"zn{j}")

                for h in range(H):
                    j, off = h // 2, (h % 2) * D
                    z_ps = zps.tile([D + 1, QW], F32, tag="z", name="z_ps")
                    qs = qT_sb[j][s]
                    deferred_pv = None  # one-pair PE lookahead within the head

                    # pair list: 2s full pairs, then the diagonal tiles
                    pairs = [("full", 2 * p) for p in range(2 * s)]
                    if PACK_DIAG:
                        pairs += [("dA", 4 * s), ("dB", 4 * s + 2)]
                    else:
                        pairs += [("d", 4 * s + i) for i in range(4)]

                    npairs = len(pairs)
                    for pi, (kind, t0) in enumerate(pairs):
                        sc = scps.tile([P, 2, QW], F32, tag="sc", name="sc")
                        ex = expp.tile([P, 2, QW], MMDT, tag="ex", name="ex")
                        if kind == "full":
                            for o in (0, 1):
                                nc.tensor.matmul(
                                    sc[:, o, :], kslice(j, off, t0 + o),
                                    qs[off : off + D, :], start=True, stop=True,
                                )
                            nc.scalar.activation(
                                ex[:], sc[:],
                                mybir.ActivationFunctionType.Exp, scale=0.125,
                            )
                            pv = [(t0, ex[:, 0, :], 0), (t0 + 1, ex[:, 1, :], 0)]
                        elif kind == "dA":
                            # d0: q cols [0:512); d1: q cols [128:512) packed
                            # into the second bank at offset 0
                            nc.tensor.matmul(
                                sc[:, 0, :], kslice(j, off, t0),
                                qs[off : off + D, :], start=True, stop=True,
                            )
                            nc.tensor.matmul(
                                sc[:, 1, 0:384], kslice(j, off, t0 + 1),
                                qs[off : off + D, P:QW], start=True, stop=True,
                            )
                            nc.scalar.activation(
                                ex[:, 0, :], sc[:, 0, :],
                                mybir.ActivationFunctionType.Exp, scale=0.125,
                            )
                            nc.scalar.activation(
                                ex[:, 1, 0:384], sc[:, 1, 0:384],
                                mybir.ActivationFunctionType.Exp, scale=0.125,
                            )
                            MASKENG.tensor_mul(ex[:, 0, 0:P], ex[:, 0, 0:P], tri_sb[:])
                            MASKENG.tensor_mul(ex[:, 1, 0:P], ex[:, 1, 0:P], tri_sb[:])
                            pv = [(t0, ex[:, 0, :], 0), (t0 + 1, ex[:, 1, 0:384], P)]
                        elif kind == "dB":
                            # d2: q cols [256:512); d3: q cols [384:512) packed
                            # behind it in the same bank
                            nc.tensor.matmul(
                                sc[:, 0, 0:256], kslice(j, off, t0),
                                qs[off : off + D, 2 * P : QW], start=True, stop=True,
                            )
                            nc.tensor.matmul(
                                sc[:, 0, 256:384], kslice(j, off, t0 + 1),
                                qs[off : off + D, 3 * P : QW], start=True, stop=True,
                            )
                            nc.scalar.activation(
                                ex[:, 0, 0:384], sc[:, 0, 0:384],
                                mybir.ActivationFunctionType.Exp, scale=0.125,
                            )
                            MASKENG.tensor_mul(ex[:, 0, 0:P], ex[:, 0, 0:P], tri_sb[:])
                            MASKENG.tensor_mul(ex[:, 0, 256:384], ex[:, 0, 256:384], tri_sb[:])
                            pv = [(t0, ex[:, 0, 0:256], 2 * P), (t0 + 1, ex[:, 0, 256:384], 3 * P)]
                        else:  # single diagonal tile (baseline style)
                            li = (t0 - 4 * s) * P
                            nc.tensor.matmul(
                                sc[:, 0, li:QW], kslice(j, off, t0),
                                qs[off : off + D, li:QW], start=True, stop=True,
                            )
                            nc.scalar.activation(
                                ex[:, 0, li:QW], sc[:, 0, li:QW],
                                mybir.ActivationFunctionType.Exp, scale=0.125,
                            )
                            MASKENG.tensor_mul(ex[:, 0, li : li + P], ex[:, 0, li : li + P], tri_sb[:])
                            pv = [(t0, ex[:, 0, li:QW], li)]

                        if deferred_pv is not None:
                            for tt, rhs, qoff, first, last in deferred_pv:
                                nc.tensor.matmul(
                                    z_ps[:, qoff:QW] if qoff else z_ps[:],
                                    vext_sb[h][:, tt, :], rhs,
                                    start=first, stop=last,
                                )
                        is_last_pair = pi == npairs - 1
                        deferred_pv = [
                            (e[0], e[1], e[2], e[0] == 0,
                             is_last_pair and (i == len(pv) - 1))
                            for i, e in enumerate(pv)
                        ]

                    # flush the last pair's PV, then normalize
                    for tt, rhs, qoff, first, last in deferred_pv:
                        nc.tensor.matmul(
                            z_ps[:, qoff:QW] if qoff else z_ps[:],
                            vext_sb[h][:, tt, :], rhs,
                            start=first, stop=last,
                        )

                    # normalize: zn = z[0:64] * (1 / z[64]); approx recip is
                    # exact to ~4e-6, far below bf16 input rounding
                    dcp = rcpp.tile([1, QW], F32, tag="dcp", name="dcp")
                    nc.vector.tensor_copy(dcp[:], z_ps[D : D + 1, :])
                    rcp = rcpp.tile([1, QW], F32, tag="rcp", name="rcp")
                    nc.vector.reciprocal_approx_fast(rcp[:], dcp[:])
                    rb_sb = rcpp.tile([D, QW], F32, tag="rb_sb", name="rb_sb")
                    if USE_GPS_BCAST:
                        nc.gpsimd.partition_broadcast(rb_sb[:], rcp[:], channels=D)
                    else:
                        # broadcast 1/denom across the 64 d-partitions via a
                        # DRAM bounce: DRAM sources allow a step-0 partition dim
                        rdr = dramp.tile([1, QW], F32, tag="rdr", name="rdr")
                        nc.sync.dma_start(rdr[:], rcp[:])
                        nc.sync.dma_start(
                            rb_sb[:],
                            bass.AP(tensor=rdr.tensor, offset=rdr.offset,
                                    ap=[[0, D]] + [list(a) for a in rdr.ap][1:]),
                        )
                    nc.vector.tensor_mul(
                        zn_sb[(s, j)][off : off + D, :], z_ps[0:D, :], rb_sb[:]
                    )

                    # PE filler at head boundaries: previous strip's projection
                    if s > 0:
                        if h == 1:
                            emit_proj_units(s - 1, [(0, 0), (0, 1), (1, 0), (1, 1)])
                        elif h == 2:
                            emit_proj_units(s - 1, [(2, 0), (2, 1), (3, 0), (3, 1)])

            # tail: last strip's projection
            emit_proj_units(NSTRIP - 1, [(qb, mt) for qb in range(4) for mt in range(2)])

    nc.finalize()
    return nc


def _get_program():
    global _PROGRAM, _PROGRAM_KEY
    key = (PACK_DIAG, USE_GPS_MASK, USE_GPS_BCAST, USE_BF16_OUT)
    if _PROGRAM is None or _PROGRAM_KEY != key:
        _PROGRAM = build_program()
        _PROGRAM_KEY = key
    return _PROGRAM


def make_in_maps(q, k, v, W_O, n_cores=N_CORES):
    """Shard full inputs into per-core maps (core = batch*4 + head_group)."""
    import ml_dtypes
    mmdt = ml_dtypes.bfloat16
    q = np.ascontiguousarray(np.asarray(q, dtype=np.float32))
    k = np.ascontiguousarray(np.asarray(k, dtype=np.float32))
    v = np.ascontiguousarray(np.asarray(v, dtype=np.float32))
    W_O = np.ascontiguousarray(np.asarray(W_O, dtype=np.float32))
    B = q.shape[0]
    qT = np.ascontiguousarray(q.reshape(B, S, N_HEADS, D).transpose(0, 2, 3, 1))
    kT = np.ascontiguousarray(k.reshape(B, S, N_HEADS, D).transpose(0, 2, 3, 1))
    # v: [b, h, S, d] -> [b, h, p(kv%128), (kvtile(16) d)]
    vh = v.reshape(B, S, N_HEADS, D).transpose(0, 2, 1, 3)
    vh = vh.reshape(B, N_HEADS, 16, P, D).transpose(0, 1, 3, 2, 4)
    vh = np.ascontiguousarray(vh.reshape(B, N_HEADS, P, 16 * D))
    # mask[kv, q] = 1 iff kv <= q  (scores live transposed: partition=kv, free=q)
    tri = np.ascontiguousarray(np.triu(np.ones((P, P), dtype=np.float32)))
    in_maps = []
    for core in range(n_cores):
        b, g = core // 4, core % 4
        hs = slice(H * g, H * (g + 1))
        in_maps.append(
            {
                "qT": np.ascontiguousarray(qT[b, hs]).astype(mmdt),
                "kT": np.ascontiguousarray(kT[b, hs]).astype(mmdt),
                "v": np.ascontiguousarray(vh[b, hs]).astype(mmdt),
                "wo": np.ascontiguousarray(W_O[hs].reshape(2 * P, D_MODEL)).astype(mmdt),
                "tri": tri.astype(mmdt),
            }
        )
    return in_maps


def kernel(residual, q, k, v, W_O, b_O, _trace=False, _trace_kwargs=None):
    global LAST_RESULTS
    residual = np.asarray(residual, dtype=np.float32)
    B = residual.shape[0]
    in_maps = make_in_maps(q, k, v, W_O)
    nc = _get_program()
    res = run_bass_kernel_spmd(
        nc, in_maps, list(range(N_CORES)), trace=_trace, **(_trace_kwargs or {})
    )
    LAST_RESULTS = res
    out = np.zeros((B, S, D_MODEL), dtype=np.float64)
    for core in range(N_CORES):
        out[core // 4] += res.results[core]["pout"].astype(np.float64)
    out += np.asarray(b_O, dtype=np.float64)
    return (residual, out.astype(np.float32))
